# revision 5
# baseline (speedup 1.0000x reference)
"""Trainium2 Bass kernel for the DNA GNN (nn_DNA_65360812310552).

Strategy (8 NeuronCores, SPMD):
  - Nodes padded to NP=10240, sharded by col-range: core c owns nodes
    [c*1280, (c+1)*1280) and ALL edges whose target (col) lies in that
    range.  Aggregation is core-local: no reduce collectives.
  - All static graph data is HOST-precomputed: deg/dis (gcn norm), the
    per-tile segment-sum selection matrices S'' (bf16, with
    dis[row]*dis[col] folded in, SBUF-resident), and the bias-path
    vector st' = dis_i * segsum(dis[row]).  The device does no prep
    passes and no dis AllGather.
  - Algebra: bk cancels in softmax; Wk is folded into the query
    (qt = glinT(glin(x,Wq)+bq, Wk)/sqrt(CH)); Wv+bv are deferred past
    attention + segment-sum (linearity).
  - Edge phase per 128-node window, in chunks of <=NB 128-edge tiles,
    with ALL layer-slices batched per DVE op:
      P   = xga * qg            (bf16, 2x mode)
      sc  = tree-reduce_CH(P)   (3 bf16 stages + 1 f32, strided out)
      ex  = exp(sc)             (ACT)
      den/rec/attn              (small DVE ops)
      EXB = broadcast_CH(attn)  (ACT, bf16 out)
      MSG = xga * EXB           (bf16, 2x mode)
      psum[window] += S''_t^T @ MSG[t,j]   (PE, S'' stationary per tile)
  - Node-table AllGather per layer is split in two halves so the first
    half overlaps the last 5 windows' compute; gather row indices are
    host-remapped to the split layout.

Self-contained: hardcodes shapes; builds the Bass program per input
(edge partition sizes baked in), runs via run_bass_kernel_spmd on
cores 0-7, reassembles the full [10000, 16] output.
"""

import numpy as np

import concourse.bacc as bacc
import concourse.bass as bass
import concourse.mybir as mybir
import concourse.tile as tile
from concourse.bass_utils import run_bass_kernel_spmd
from concourse.masks import make_identity

# problem constants
N = 10000
E = 160000
C = 128
H = 8
CH = 16
G = 16
CG = 8
L = 5
NF = 14
NFP = 16          # NF padded
DOUT = 16
NCORES = 8

NP = 10240        # padded node count = 8 * 1280
NSL = NP // NCORES  # 1280 nodes per core
NW = NSL // 128     # 10 windows of 128 nodes per core
NHALF = NSL // 2    # AG split point (640 = windows 0-4)
NB = 8              # max tiles (of 128 edges) per chunk

F32 = mybir.dt.float32
BF16 = mybir.dt.bfloat16
I16 = mybir.dt.int16

EDT = mybir.dt.bfloat16   # edge-pipeline dtype


def _wrap_idx(a: np.ndarray) -> np.ndarray:
    """[T] int -> [128, T//16] int16 in dma_gather's wrapped layout:
    idx j lives at partition j%16, column j//16, replicated 8x."""
    T = a.shape[0]
    assert T % 16 == 0
    w = a.reshape(T // 16, 16).T.astype(np.int16)  # [16, T//16]
    return np.tile(w, (8, 1))                       # [128, T//16]


def _chunks(nt: int) -> list[int]:
    k = -(-nt // NB)
    base = nt // k
    out = [base] * k
    for i in range(nt - base * k):
        out[i] += 1
    return out


def _remap_row(n):
    """Global node id -> row in the split-AllGather table layout.
    AG half1 gathers local rows [0,640) of all cores into table rows
    [0, 5120); half2 gathers [640,1280) into [5120, 10240)."""
    c = n // NSL
    o = n % NSL
    return np.where(o < NHALF, c * NHALF + o, NCORES * NHALF + c * NHALF + (o - NHALF))


def build_program(tiles_w: list[int], skip=frozenset(), reps=1):
    """Build the SPMD Bass program.  tiles_w[w] = number of 128-edge
    tiles in window w (identical across cores, host-padded)."""
    TOT = sum(tiles_w) * 128          # padded edges per core
    NTIL = sum(tiles_w)

    nc = bacc.Bacc("TRN2", target_bir_lowering=False, debug=False,
                   num_devices=NCORES)

    # ---- I/O ----
    xsl = nc.dram_tensor("xsl", [NSL, NFP], F32, kind="ExternalInput")
    rowi = nc.dram_tensor("rowi", [128, TOT // 16], I16, kind="ExternalInput")
    coli = nc.dram_tensor("coli", [128, TOT // 16], I16, kind="ExternalInput")
    ssp_d = nc.dram_tensor("ssp", [128, NTIL * 128], BF16, kind="ExternalInput")
    strow_d = nc.dram_tensor("strow", [1, NSL], F32, kind="ExternalInput")
    w1_d = nc.dram_tensor("w1", [NFP, C], F32, kind="ExternalInput")
    b1_d = nc.dram_tensor("b1", [C], F32, kind="ExternalInput")
    wq_d = nc.dram_tensor("wq", [L, C, C], F32, kind="ExternalInput")
    wkt_d = nc.dram_tensor("wkt", [L, C, C], F32, kind="ExternalInput")
    wv_d = nc.dram_tensor("wv", [L, C, C], F32, kind="ExternalInput")
    bq_d = nc.dram_tensor("bq", [L, C], F32, kind="ExternalInput")
    bv_d = nc.dram_tensor("bv", [L, C], F32, kind="ExternalInput")
    l2w_d = nc.dram_tensor("l2w", [C, DOUT], F32, kind="ExternalInput")
    l2b_d = nc.dram_tensor("l2b", [DOUT], F32, kind="ExternalInput")
    y_d = nc.dram_tensor("y", [NSL, DOUT], F32, kind="ExternalOutput")

    # ---- internal DRAM ----
    xsl_d = nc.dram_tensor("xsl_int", [NSL, C], EDT)     # AG input (x_l slice)
    qsl_d = nc.dram_tensor("qsl_int", [NSL, C], EDT)     # qt table (local cols)
    xf_b = nc.dram_tensor("xf_b", [NP, C], EDT, addr_space="Shared")
    tq_f = nc.dram_tensor("tq_f", [NP, L * C], EDT)      # packed x0..x4 rows

    groups = [list(range(NCORES))]
    HROWS = NCORES * NHALF   # 5120

    with tile.TileContext(nc) as tc:
        with (
            tc.tile_pool(name="const", bufs=1) as cpool,
            tc.tile_pool(name="work", bufs=2) as pool,
            tc.tile_pool(name="psum", bufs=3, space="PSUM") as psp,
            tc.tile_pool(name="psw", bufs=2, space="PSUM") as pswp,
        ):
            # ---------- constants ----------
            ident = cpool.tile([128, 128], F32)
            make_identity(nc, ident[:])

            w1_sb = cpool.tile([NFP, C], F32)
            nc.sync.dma_start(out=w1_sb[:], in_=w1_d[:])
            b1_sb = cpool.tile([C, 1], F32)
            nc.sync.dma_start(out=b1_sb[:], in_=b1_d[:, None])
            l2w_sb = cpool.tile([C, DOUT], F32)
            nc.sync.dma_start(out=l2w_sb[:], in_=l2w_d[:])
            l2b_sb = cpool.tile([1, DOUT], F32)
            nc.sync.dma_start(out=l2b_sb[:], in_=l2b_d[:][None, :])
            ones_row = cpool.tile([1, 128], F32)
            nc.gpsimd.memset(ones_row[:], 1.0)

            rowi_sb = cpool.tile([128, TOT // 16], I16)
            nc.sync.dma_start(out=rowi_sb[:], in_=rowi[:])
            coli_sb = cpool.tile([128, TOT // 16], I16)
            nc.sync.dma_start(out=coli_sb[:], in_=coli[:])
            ssp_sb = cpool.tile([128, NTIL, 128], BF16)
            nc.sync.dma_start(
                out=ssp_sb[:], in_=ssp_d[:].rearrange("p (t n) -> p t n", t=NTIL))
            strow_sb = cpool.tile([1, NSL], F32)
            nc.sync.dma_start(out=strow_sb[:], in_=strow_d[:])

            xc_all = cpool.tile([128, NSL], F32)    # current x_l, c-major

            def ag_table(lslice):
                """Split AllGather of xsl_d into xf_b, then copy into the
                packed table column block for layer-slice `lslice`."""
                if "cc" not in skip:
                    nc.gpsimd.collective_compute(
                        "AllGather", mybir.AluOpType.bypass,
                        replica_groups=groups,
                        ins=[xsl_d[0:NHALF]], outs=[xf_b[0:HROWS]])
                nc.sync.dma_start(out=tq_f[0:HROWS, lslice * C:(lslice + 1) * C],
                                  in_=xf_b[0:HROWS])

            def ag_table2(lslice):
                if "cc" not in skip:
                    nc.gpsimd.collective_compute(
                        "AllGather", mybir.AluOpType.bypass,
                        replica_groups=groups,
                        ins=[xsl_d[NHALF:NSL]], outs=[xf_b[HROWS:NP]])
                nc.sync.dma_start(out=tq_f[HROWS:NP, lslice * C:(lslice + 1) * C],
                                  in_=xf_b[HROWS:NP])

            for _rep in range(reps):
              # ---------- x0 = relu(x @ W1 + b1) ----------
              for w in range(NW):
                  xin = pool.tile([128, NFP], F32, tag="xin")
                  nc.sync.dma_start(out=xin[:],
                                    in_=xsl[w * 128:(w + 1) * 128, :])
                  pxt = psp.tile([NFP, 128], F32, tag="pnt")
                  nc.tensor.transpose(pxt[:], xin[:], ident[:])
                  xt = pool.tile([NFP, 128], F32, tag="xt")
                  nc.scalar.copy(xt[:], pxt[:])
                  pm = psp.tile([128, 128], F32, tag="pm")
                  nc.tensor.matmul(pm[:], lhsT=w1_sb[:], rhs=xt[:],
                                   start=True, stop=True)
                  nc.scalar.activation(
                      out=xc_all[:, w * 128:(w + 1) * 128], in_=pm[:],
                      func=mybir.ActivationFunctionType.Relu,
                      bias=b1_sb[:, 0:1])
                  pnt = psp.tile([128, 128], F32, tag="pnt")
                  nc.tensor.transpose(pnt[:], xc_all[:, w * 128:(w + 1) * 128],
                                      ident[:])
                  xn = pool.tile([128, C], EDT, tag="xn")
                  nc.scalar.copy(xn[:], pnt[:])
                  nc.sync.dma_start(out=xsl_d[w * 128:(w + 1) * 128, :],
                                    in_=xn[:])
                  if w == NW // 2 - 1:
                      ag_table(0)
              ag_table2(0)

              # ---------- layers ----------
              for l in range(L):
                  Lc = l + 1
                  wq_sb = pool.tile([128, 128], F32, tag="wq_sb")
                  nc.sync.dma_start(out=wq_sb[:], in_=wq_d[l])
                  wkt_sb = pool.tile([128, 128], F32, tag="wkt_sb")
                  nc.sync.dma_start(out=wkt_sb[:], in_=wkt_d[l])
                  wv_sb = pool.tile([128, 128], F32, tag="wv_sb")
                  nc.sync.dma_start(out=wv_sb[:], in_=wv_d[l])
                  bq_sb = pool.tile([C, 1], F32, tag="bq_sb")
                  nc.sync.dma_start(out=bq_sb[:], in_=bq_d[l][:, None])
                  bv_row = pool.tile([1, C], F32, tag="bv_row")
                  nc.sync.dma_start(out=bv_row[:], in_=bv_d[l][None, :])

                  # qt = glinT(glin(x_l, Wq)+bq, Wk) / 4, from xc_all (c-major)
                  for ch in range(NW):
                      pq = psp.tile([128, 128], F32, tag="pm")
                      nc.tensor.matmul(pq[:], lhsT=wq_sb[:],
                                       rhs=xc_all[:, ch * 128:(ch + 1) * 128],
                                       start=True, stop=True)
                      qs = pool.tile([128, 128], F32, tag="qs")
                      nc.scalar.activation(
                          out=qs[:], in_=pq[:],
                          func=mybir.ActivationFunctionType.Identity,
                          bias=bq_sb[:, 0:1])
                      pq2 = psp.tile([128, 128], F32, tag="pm")
                      nc.tensor.matmul(pq2[:], lhsT=wkt_sb[:], rhs=qs[:],
                                       start=True, stop=True)
                      qtc = pool.tile([128, 128], F32, tag="qtc")
                      nc.scalar.activation(
                          out=qtc[:], in_=pq2[:],
                          func=mybir.ActivationFunctionType.Copy, scale=0.25)
                      pq3 = psp.tile([128, 128], F32, tag="pnt")
                      nc.tensor.transpose(pq3[:], qtc[:], ident[:])
                      qn = pool.tile([128, C], EDT, tag="xn")
                      nc.scalar.copy(qn[:], pq3[:])
                      nc.sync.dma_start(out=qsl_d[ch * 128:(ch + 1) * 128, :],
                                        in_=qn[:])

                  # ---- edge phase + per-window aggregation + dense ----
                  t0 = 0
                  for w in range(NW):
                      upsw = pswp.tile([128, C], F32, tag="acc")
                      first = True
                      wtiles = tiles_w[w]
                      wend = t0 + wtiles
                      for nb in _chunks(wtiles):
                          qg = pool.tile([128, NB, C], EDT, tag="qg")
                          xga = pool.tile([128, NB, Lc * C], EDT, tag="xga")
                          if "gather" in skip:
                              nc.vector.memset(qg[:, :nb, :], 0.25)
                              nc.vector.memset(xga[:, :nb, :], 0.25)
                          else:
                              nc.gpsimd.dma_gather(
                                  qg[:, :nb, :], qsl_d[:],
                                  coli_sb[:, t0 * 8:(t0 + nb) * 8],
                                  nb * 128, nb * 128, C)
                              nc.gpsimd.dma_gather(
                                  xga[:, :nb, :], tq_f[:, :Lc * C],
                                  rowi_sb[:, t0 * 8:(t0 + nb) * 8],
                                  nb * 128, nb * 128, Lc * C,
                                  elem_step=L * C)
                          MSG = pool.tile([128, NB, Lc, C], EDT, tag="MSG")
                          if "dve" in skip:
                              nc.vector.memset(MSG[:, :nb, :, :], 0.25)
                          else:
                              xv = xga[:, :nb, :].rearrange(
                                  "p b (l c) -> p b l c", l=Lc)
                              # P = xga * qg (broadcast over slices; 2x)
                              P = pool.tile([128, NB, Lc, C], EDT, tag="P")
                              nc.vector.tensor_tensor(
                                  out=P[:, :nb], in0=xv,
                                  in1=qg[:, :nb, :].unsqueeze(2).to_broadcast(
                                      [128, nb, Lc, C]),
                                  op=mybir.AluOpType.mult)
                              # tree reduce over CH: 3 bf16 stages + f32 tail
                              Ph = P[:, :nb].rearrange(
                                  "p b l (h c) -> p b l h c", h=H)
                              T1 = pool.tile([128, NB, Lc, H, 8], EDT, tag="T1")
                              nc.vector.tensor_tensor(
                                  out=T1[:, :nb], in0=Ph[:, :, :, :, 0:8],
                                  in1=Ph[:, :, :, :, 8:16],
                                  op=mybir.AluOpType.add)
                              T2 = pool.tile([128, NB, Lc, H, 4], EDT, tag="T2")
                              nc.vector.tensor_tensor(
                                  out=T2[:, :nb],
                                  in0=T1[:, :nb, :, :, 0:4],
                                  in1=T1[:, :nb, :, :, 4:8],
                                  op=mybir.AluOpType.add)
                              T3 = pool.tile([128, NB, Lc, H, 2], EDT, tag="T3")
                              nc.vector.tensor_tensor(
                                  out=T3[:, :nb],
                                  in0=T2[:, :nb, :, :, 0:2],
                                  in1=T2[:, :nb, :, :, 2:4],
                                  op=mybir.AluOpType.add)
                              sc = pool.tile([128, NB, H, Lc], F32, tag="sc")
                              nc.vector.tensor_tensor(
                                  out=sc[:, :nb].rearrange(
                                      "p b h l -> p b l h"),
                                  in0=T3[:, :nb, :, :, 0],
                                  in1=T3[:, :nb, :, :, 1],
                                  op=mybir.AluOpType.add)
                              # softmax over slices
                              ex = pool.tile([128, NB, H, Lc], F32, tag="ex")
                              nc.scalar.activation(
                                  out=ex[:, :nb], in_=sc[:, :nb],
                                  func=mybir.ActivationFunctionType.Exp)
                              den = pool.tile([128, NB, H], F32, tag="den")
                              nc.vector.reduce_sum(out=den[:, :nb, :],
                                                   in_=ex[:, :nb],
                                                   axis=mybir.AxisListType.X)
                              rec = pool.tile([128, NB, H], F32, tag="rec")
                              nc.vector.reciprocal(rec[:, :nb, :], den[:, :nb, :])
                              attn = pool.tile([128, NB, Lc, H], F32, tag="attn")
                              nc.vector.tensor_tensor(
                                  out=attn[:, :nb].rearrange(
                                      "p b l h -> p b h l"),
                                  in0=ex[:, :nb],
                                  in1=rec[:, :nb, :].unsqueeze(3).to_broadcast(
                                      [128, nb, H, Lc]),
                                  op=mybir.AluOpType.mult)
                              # EXB = attn broadcast over CH (ACT, bf16 out)
                              EXB = pool.tile([128, NB, Lc, H, CH], EDT, tag="EXB")
                              nc.scalar.activation(
                                  out=EXB[:, :nb],
                                  in_=attn[:, :nb].unsqueeze(
                                      4).to_broadcast([128, nb, Lc, H, CH]),
                                  func=mybir.ActivationFunctionType.Copy)
                              # MSG = xga * EXB (2x)
                              nc.vector.tensor_tensor(
                                  out=MSG[:, :nb], in0=xv,
                                  in1=EXB[:, :nb].rearrange(
                                      "p b l h c -> p b l (h c)"),
                                  op=mybir.AluOpType.mult)
                          if "pe" not in skip:
                              for t in range(nb):
                                  for j in range(Lc):
                                      nc.tensor.matmul(
                                          upsw[:], lhsT=ssp_sb[:, t0 + t, :],
                                          rhs=MSG[:, t, j, :],
                                          start=first,
                                          stop=(t == nb - 1 and j == Lc - 1 and
                                                t0 + nb >= wend))
                                      first = False
                          t0 += nb

                      # ---- dense epilogue for this window ----
                      uw = pool.tile([128, C], F32, tag="uw")
                      nc.scalar.copy(uw[:], upsw[:])
                      put = psp.tile([128, C], F32, tag="pnt")
                      nc.tensor.transpose(put[:], uw[:], ident[:])
                      uc = pool.tile([128, C], F32, tag="uc")
                      nc.scalar.copy(uc[:], put[:])
                      pg = psp.tile([128, C], F32, tag="pm")
                      nc.tensor.matmul(pg[:], lhsT=wv_sb[:], rhs=uc[:],
                                       start=True, stop=False)
                      nc.tensor.matmul(pg[:], lhsT=bv_row[:],
                                       rhs=strow_sb[:, w * 128:(w + 1) * 128],
                                       start=False, stop=True)
                      # xl = relu(...)  (c-major, directly into xc_all)
                      nc.scalar.activation(
                          out=xc_all[:, w * 128:(w + 1) * 128], in_=pg[:],
                          func=mybir.ActivationFunctionType.Relu)
                      if l < L - 1:
                          pnt = psp.tile([128, C], F32, tag="pnt")
                          nc.tensor.transpose(
                              pnt[:], xc_all[:, w * 128:(w + 1) * 128], ident[:])
                          xne = pool.tile([128, C], EDT, tag="xn")
                          nc.scalar.copy(xne[:], pnt[:])
                          nc.sync.dma_start(
                              out=xsl_d[w * 128:(w + 1) * 128, :], in_=xne[:])
                          if w == NW // 2 - 1:
                              ag_table(l + 1)
                  if l < L - 1:
                      ag_table2(l + 1)

              # ---------- output: y = x5 @ l2w + l2b ----------
              for ch in range(NW):
                  py = psp.tile([128, DOUT], F32, tag="pm")
                  nc.tensor.matmul(py[:], lhsT=xc_all[:, ch * 128:(ch + 1) * 128],
                                   rhs=l2w_sb[:], start=True, stop=False)
                  nc.tensor.matmul(py[:], lhsT=ones_row[:], rhs=l2b_sb[:],
                                   start=False, stop=True)
                  ysb = pool.tile([128, DOUT], F32, tag="ysb")
                  nc.scalar.copy(ysb[:], py[:])
                  nc.sync.dma_start(out=y_d[ch * 128:(ch + 1) * 128, :],
                                    in_=ysb[:])

    nc.compile()
    return nc


def _prep_host(x, edge_index):
    """Shard + sort edges, build per-core index inputs."""
    row = np.concatenate([np.asarray(edge_index[0]), np.arange(N)]).astype(np.int64)
    col = np.concatenate([np.asarray(edge_index[1]), np.arange(N)]).astype(np.int64)

    core = col // NSL
    counts = np.zeros((NCORES, NW), dtype=np.int64)
    per_core = []
    for c in range(NCORES):
        m = core == c
        rc, cc = row[m], col[m]
        o = np.argsort(cc, kind="stable")
        rc, cc = rc[o], cc[o]
        per_core.append((rc, cc))
        lw = (cc - c * NSL) // 128
        for w in range(NW):
            counts[c, w] = int((lw == w).sum())
    tiles_w = [int(np.ceil(counts[:, w].max() / 128)) for w in range(NW)]
    TOT = sum(tiles_w) * 128

    # gcn norm (host): deg over targets incl self-loops; pad nodes get
    # deg=1 (dis=1) but never appear in any edge, so they contribute 0.
    deg = np.bincount(col, minlength=NP).astype(np.float64)
    deg[N:] = 1.0
    dis = 1.0 / np.sqrt(deg)

    rows_p = np.zeros((NCORES, TOT), dtype=np.int64)     # remapped table rows
    cols_p = np.zeros((NCORES, TOT), dtype=np.int64)     # local col idx
    sval_p = np.zeros((NCORES, TOT), dtype=np.float32)   # dis[row]*dis[col]
    nloc_p = np.zeros((NCORES, TOT), dtype=np.int64)     # col within window
    strow = np.zeros((NCORES, NSL), dtype=np.float32)    # dis_i*segsum(dis[row])
    for c in range(NCORES):
        rc, cc = per_core[c]
        lw = (cc - c * NSL) // 128
        pos = 0
        for w in range(NW):
            m = lw == w
            k = int(m.sum())
            rows_p[c, pos:pos + k] = _remap_row(rc[m])
            cols_p[c, pos:pos + k] = cc[m] - c * NSL
            sval_p[c, pos:pos + k] = (dis[rc[m]] * dis[cc[m]]).astype(np.float32)
            nloc_p[c, pos:pos + k] = cc[m] - c * NSL - w * 128
            pos += tiles_w[w] * 128
        lo = np.zeros(NSL, dtype=np.float64)
        np.add.at(lo, cc - c * NSL, dis[rc])
        strow[c] = (dis[c * NSL:(c + 1) * NSL] * lo).astype(np.float32)

    # S'' selection matrices, host-swizzled to [128, NTIL*128] bf16:
    # ssp[p, t*128+n] = (nloc(e)==n) * sval(e) for edge e = t*128+p,
    # 0 for pad slots (sval=0 there).
    NTIL = TOT // 128
    import ml_dtypes
    ssp = np.zeros((NCORES, 128, NTIL * 128), dtype=ml_dtypes.bfloat16)
    for c in range(NCORES):
        nl = nloc_p[c].reshape(NTIL, 128)     # [t, p]
        sv = sval_p[c].reshape(NTIL, 128)
        t_i, p_i = np.nonzero(sv != 0.0)
        ssp[c, p_i, t_i * 128 + nl[t_i, p_i]] = sv[t_i, p_i].astype(
            ml_dtypes.bfloat16)

    return tiles_w, rows_p, cols_p, ssp, strow


def prepare(inputs):
    return _prepare_impl(inputs)


def _prepare_impl(inputs):
    x = np.asarray(inputs["x"], dtype=np.float32)
    edge_index = np.asarray(inputs["edge_index"])
    lin1_w = np.asarray(inputs["lin1_w"], dtype=np.float32)
    lin1_b = np.asarray(inputs["lin1_b"], dtype=np.float32)
    Wq = np.asarray(inputs["Wq"], dtype=np.float32)
    bq = np.asarray(inputs["bq"], dtype=np.float32)
    Wk = np.asarray(inputs["Wk"], dtype=np.float32)
    Wv = np.asarray(inputs["Wv"], dtype=np.float32)
    bv = np.asarray(inputs["bv"], dtype=np.float32)
    lin2_w = np.asarray(inputs["lin2_w"], dtype=np.float32)
    lin2_b = np.asarray(inputs["lin2_b"], dtype=np.float32)

    tiles_w, rows_p, cols_p, ssp, strow = _prep_host(x, edge_index)
    nc = build_program(tiles_w)

    def blockdiag(W):  # W [G, CG, CG] -> [C, C]
        out = np.zeros((C, C), dtype=np.float32)
        for g in range(G):
            out[g * CG:(g + 1) * CG, g * CG:(g + 1) * CG] = W[g]
        return out

    wq_bd = np.stack([blockdiag(Wq[l]) for l in range(L)])
    wkt_bd = np.stack([blockdiag(Wk[l].transpose(0, 2, 1)) for l in range(L)])
    wv_bd = np.stack([blockdiag(Wv[l]) for l in range(L)])

    x_pad = np.zeros((NP, NFP), dtype=np.float32)
    x_pad[:N, :NF] = x
    w1_pad = np.zeros((NFP, C), dtype=np.float32)
    w1_pad[:NF] = lin1_w

    in_maps = []
    for c in range(NCORES):
        in_maps.append({
            "xsl": x_pad[c * NSL:(c + 1) * NSL],
            "rowi": _wrap_idx(rows_p[c]),
            "coli": _wrap_idx(cols_p[c]),
            "ssp": np.ascontiguousarray(ssp[c]),
            "strow": strow[c][None, :],
            "w1": w1_pad,
            "b1": lin1_b,
            "wq": wq_bd,
            "wkt": wkt_bd,
            "wv": wv_bd,
            "bq": bq,
            "bv": bv,
            "l2w": lin2_w,
            "l2b": lin2_b,
        })

    return nc, in_maps


def assemble(res) -> np.ndarray:
    y = np.concatenate([res.results[c]["y"] for c in range(NCORES)], axis=0)
    return np.ascontiguousarray(y[:N]).astype(np.float32)


def kernel(**inputs) -> np.ndarray:
    nc, in_maps = _prepare_impl(inputs)
    res = run_bass_kernel_spmd(nc, in_maps, list(range(NCORES)))
    global LAST_RESULTS
    LAST_RESULTS = res
    return assemble(res)


LAST_RESULTS = None


if __name__ == "__main__":
    import reference
    inp = {k: np.asarray(v) for k, v in reference.setup_inputs().items()}
    out = kernel(**inp)
    print(out.shape, out.dtype)
